# revision 24
# baseline (speedup 1.0000x reference)
"""Entmax-1.5 forward (last-axis, d=1024) as a Bass/Tile kernel for 8 TRN2 cores.

Algorithm (no sort / no cumsum):
  The entmax-1.5 output is Y = ((x - T)/2)_+^2 where the threshold T is the
  unique root of f(T) = sum_j (x_j - T)_+^2 = 4 (raw-logit space; this is the
  reference's tau_star mapped back through the max-shift and *0.5 scaling).
  f is strictly decreasing and piecewise quadratic, so T is found per-row with
  3 "active-set" iterations (solve the local quadratic exactly on the current
  support mask, mirroring the reference's clip(delta, 0) semantics), then one
  Newton polish step:

    stats at T:   A  = sum max(x, T)        -> S1 = A - d*T = sum (x-T)_+
                  S2 = sum (x-T)_+^2
    quasi-Newton: T += (S2 - 2*sqrt(S2)) / S1
                  (the exact active-set solve with curvature estimated via
                   Cauchy-Schwarz S0 ~= S1^2/S2 -- needs no mask-count pass;
                   exact for uniform masks, fixed point at S2=4, first-order
                   identical to Newton at the root)
    newton:       T += (S2 - 4) / (2*S1)
    output:       Y = (0.5*(x - T)_+)^2

  Init: T0 = rowmax - 1.2.  (Validated on the reference inputs: worst-row
  |Y - Y_ref| ~ 6e-5.)

Sharding: 98304 rows split contiguously across 8 cores (12288 rows each);
rows are fully independent.

Engine mapping per [128, 1024] tile:
  DVE : reduce_max (init), tensor_scalar max (m=max(x,T), accum->A; 2x mode),
        output relu (sub+max fused)
  ACT : Square activation with bias=T, scale=-1 on m (accum -> S2),
        output square with scale=0.5, sqrt(S2) in the solve

Chunks of 1024 rows are emitted pairwise software-interleaved so each engine
always has an independent chunk's work adjacent in its instruction stream
(hides the per-iteration solve barrier).
"""

import numpy as np

_N_CORES = 8
_D = 1024
_P = 128
_ROWS_TOTAL = 8 * 12 * 1024            # 98304
_ROWS_PER_CORE = _ROWS_TOTAL // _N_CORES  # 12288
_TILES_PER_CORE = _ROWS_PER_CORE // _P    # 96
_CHUNK_TILES = 8                          # tiles per chunk (1024 rows)
_N_CHUNKS = _TILES_PER_CORE // _CHUNK_TILES  # 12
_N_AS = 3                                 # active-set iterations
_T0_OFFSET = -1.2                         # T0 = rowmax + _T0_OFFSET
_S0_ON_GPSIMD = False                      # run is_gt passes on GPSIMD

_CACHE = {}


def _build(reps: int = 1):
    from contextlib import ExitStack

    import concourse.bacc as bacc
    import concourse.tile as tile
    from concourse import mybir

    f32 = mybir.dt.float32
    bf16 = mybir.dt.bfloat16
    Alu = mybir.AluOpType
    Act = mybir.ActivationFunctionType
    AX = mybir.AxisListType.X

    nc = bacc.Bacc("TRN2", target_bir_lowering=False, debug=False,
                   num_devices=_N_CORES)
    x_d = nc.dram_tensor("x", (_ROWS_PER_CORE, _D), f32, kind="ExternalInput")
    y_d = nc.dram_tensor("y", (_ROWS_PER_CORE, _D), f32, kind="ExternalOutput")

    # chunk c, partition p, slot t  <->  row c*1024 + p*8 + t
    # (each partition reads 8 consecutive rows = 32KB contiguous per DMA)
    x_ap = x_d.ap().rearrange("(c p t) d -> c p t d", p=_P, t=_CHUNK_TILES)
    y_ap = y_d.ap().rearrange("(c p t) d -> c p t d", p=_P, t=_CHUNK_TILES)

    with tile.TileContext(nc) as tc, ExitStack() as ctx:
        xp = ctx.enter_context(tc.tile_pool(name="xp", bufs=3))
        yp = ctx.enter_context(tc.tile_pool(name="yp", bufs=2))
        mp = ctx.enter_context(tc.tile_pool(name="mp", bufs=3))
        jp = ctx.enter_context(tc.tile_pool(name="jp", bufs=2))
        sp = ctx.enter_context(tc.tile_pool(name="sp", bufs=4))

        C = _CHUNK_TILES

        def emit_load(st, c):
            st["x"] = xp.tile([_P, C, _D], f32, tag="x", name="xchunk")
            nc.sync.dma_start(out=st["x"], in_=x_ap[c])
            for name in ("T", "rmax", "A", "S2", "S1", "u1",
                         "rec", "dlt"):
                st[name] = sp.tile([_P, C], f32, tag=name, name=name)

        def emit_init(st):
            xt, T, rmax = st["x"], st["T"], st["rmax"]
            for t in range(C):
                nc.vector.reduce_max(rmax[:, t:t + 1], xt[:, t, :], AX)
            nc.vector.tensor_scalar(T, rmax, float(_T0_OFFSET), None, Alu.add)

        def emit_stats(st):
            xt, T, A, S2 = st["x"], st["T"], st["A"], st["S2"]
            for t in range(C):
                m_t = mp.tile([_P, _D], f32, tag="m")
                junk2 = jp.tile([_P, _D], bf16, tag="junk2")
                nc.vector.tensor_scalar(
                    m_t, xt[:, t, :], T[:, t:t + 1], None,
                    Alu.max, Alu.add, accum_out=A[:, t:t + 1])
                # square((-1)*m + T) = (m - T)^2 ; zero off-mask
                nc.scalar.activation(
                    junk2, m_t, Act.Square, bias=T[:, t:t + 1],
                    scale=-1.0, accum_out=S2[:, t:t + 1])

        def emit_solve(st):
            # quasi-Newton step, S0-free: curvature from Cauchy-Schwarz
            # (S0 ~= S1^2/S2) turns the exact mask solve into
            #   T += (S2 - 2*sqrt(S2)) / S1
            # (exact for uniform masks; fixed point at S2=4; first-order
            #  identical to Newton near the root)
            T, A, S2 = st["T"], st["A"], st["S2"]
            S1, u1, rec, dlt = st["S1"], st["u1"], st["rec"], st["dlt"]
            nc.vector.scalar_tensor_tensor(
                S1, T, float(-_D), A, Alu.mult, Alu.add)       # S1 = A - d*T
            nc.vector.tensor_scalar(S1, S1, 1e-12, None, Alu.max)
            nc.scalar.activation(u1, S2, Act.Sqrt)             # sqrt(S2)
            nc.vector.scalar_tensor_tensor(
                u1, u1, -2.0, S2, Alu.mult, Alu.add)           # S2 - 2*sqrt
            nc.vector.reciprocal(rec, S1)
            nc.vector.tensor_tensor(dlt, u1, rec, Alu.mult)
            nc.vector.tensor_tensor(T, T, dlt, Alu.add)

        def emit_newton_solve(st):
            # T += (S2-4)/(2*S1)
            T, A, S2 = st["T"], st["A"], st["S2"]
            S1, u1, rec, dlt = st["S1"], st["u1"], st["rec"], st["dlt"]
            nc.vector.scalar_tensor_tensor(
                S1, T, float(-_D), A, Alu.mult, Alu.add)
            nc.vector.tensor_scalar(S1, S1, 1e-12, 2.0, Alu.max, Alu.mult)
            nc.vector.reciprocal(rec, S1)                      # 1/(2*S1)
            nc.vector.tensor_scalar(u1, S2, -4.0, None, Alu.add)
            nc.vector.tensor_tensor(dlt, u1, rec, Alu.mult)
            nc.vector.tensor_tensor(T, T, dlt, Alu.add)

        def emit_out(st, c):
            # Y = (0.5*(x - T)_+)^2
            xt, T = st["x"], st["T"]
            yt = yp.tile([_P, C, _D], f32, tag="y")
            for t in range(C):
                r_t = mp.tile([_P, _D], f32, tag="m")
                nc.vector.tensor_scalar(
                    r_t, xt[:, t, :], T[:, t:t + 1], 0.0,
                    Alu.subtract, Alu.max)
                if t < 5:
                    # (r*0.25)*r = (r/2)^2 in one DVE op; offloads ACT,
                    # which is the saturated engine
                    nc.vector.scalar_tensor_tensor(
                        yt[:, t, :], r_t, 0.25, r_t, Alu.mult, Alu.mult)
                else:
                    nc.scalar.activation(
                        yt[:, t, :], r_t, Act.Square, bias=0.0, scale=0.5)
            nc.sync.dma_start(out=y_ap[c], in_=yt)

        # Two-chunk software interleave: at every solve barrier of chunk a,
        # each engine has chunk b's independent work adjacent in its stream.
        total = _N_CHUNKS * reps
        for base in range(0, total, 2):
            ca, cb = base % _N_CHUNKS, (base + 1) % _N_CHUNKS
            sa, sb = {}, {}
            emit_load(sa, ca)
            emit_load(sb, cb)
            emit_init(sa)
            emit_init(sb)
            for it in range(_N_AS):
                emit_stats(sa)
                emit_stats(sb)
                emit_solve(sa)
                emit_solve(sb)
            emit_stats(sa)
            emit_stats(sb)
            emit_newton_solve(sa)
            emit_newton_solve(sb)
            emit_out(sa, ca)
            emit_out(sb, cb)

    nc.compile()
    return nc


def _get_nc(reps: int = 1):
    key = ("nc", reps)
    if key not in _CACHE:
        _CACHE[key] = _build(reps)
    return _CACHE[key]


def kernel(X: np.ndarray) -> np.ndarray:
    from concourse.bass_utils import run_bass_kernel_spmd

    orig_shape = tuple(X.shape)
    Xf = np.ascontiguousarray(
        np.asarray(X, dtype=np.float32).reshape(-1, _D))
    assert Xf.shape[0] == _ROWS_TOTAL, Xf.shape

    nc = _get_nc()
    in_maps = [
        {"x": Xf[i * _ROWS_PER_CORE:(i + 1) * _ROWS_PER_CORE]}
        for i in range(_N_CORES)
    ]
    res = run_bass_kernel_spmd(nc, in_maps, core_ids=list(range(_N_CORES)))
    Y = np.concatenate([r["y"] for r in res.results], axis=0)
    return Y.reshape(orig_shape)
